# revision 9
# baseline (speedup 1.0000x reference)
"""Trainium2 Bass kernel for nn_Model_1580547969651.

Math (from the reference):
    s    = x @ sum(y, axis=0)          # (B,) row-sums of x @ y^T
    h    = hardswish(s)                # s * clip(s+3, 0, 6) / 6
    out  = clip(h + noise, -0.5, 0.5)  # (B, 1)

Strategy: COLUMN-shard x and y across the 8 cores (512 features each).
Each core's column-sum of its y shard is locally complete, so there is
no mid-kernel collective. y streams in (s p c)-packed so every DMA
descriptor covers a contiguous 16KB per partition; the VectorEngine
folds each 2MB super-tile into a (128, 512) accumulator as it lands.
One ones(128,128) matmul then does the partition-sum AND the 128-way
broadcast in one shot. Phase B computes partial dots s_i = x[:, F_i] @
ysum_i for ALL 8192 rows with fused scalar_tensor_tensor ops while x
streams (host pre-permutes x rows so this layout still produces
batch-ordered partials). The partials are transposed on the
VectorEngine (32x32 blocks) so the ReduceScatter bounce DMA is
contiguous; the 32KB->4KB ReduceScatter hands each core its 1024-row
output shard, and the elementwise tail runs in a DMA-friendly (8,128)
layout. A dummy 32B AllReduce issued up front absorbs ncfw wake-up /
rendezvous costs while the streams run.
"""

import numpy as np

from concourse import bass, bacc, mybir, tile
from concourse.bass_utils import run_bass_kernel_spmd

B = 8192
F = 4096
NCORES = 8
FL = F // NCORES        # 512 features per core
BL = B // NCORES        # 1024 output rows per core
NST = 8                 # y/x super-tiles (128 part x 8 subtiles x 512)
NSUB = 8                # subtiles per super-tile
NT = NST * NSUB         # 64 (128-row) tiles covering all 8192 rows
FP32 = mybir.dt.float32

_CACHE: dict = {}


def _build():
    nc = bacc.Bacc(
        "TRN2",
        target_bir_lowering=False,
        debug=False,
        num_devices=NCORES,
    )

    x_d = nc.dram_tensor("x", [B, FL], FP32, kind="ExternalInput")
    y_d = nc.dram_tensor("y", [B, FL], FP32, kind="ExternalInput")
    nz_d = nc.dram_tensor("noise", [BL, 1], FP32, kind="ExternalInput")
    out_d = nc.dram_tensor("out", [BL, 1], FP32, kind="ExternalOutput")

    # (s p c) packing: partition p's slice of super-tile s is 8 consecutive
    # DRAM rows = one contiguous 16KB chunk per descriptor.
    y_r = y_d[:, :].rearrange("(s p c) f -> s p c f", p=128, c=NSUB)
    x_r = x_d[:, :].rearrange("(s p c) f -> s p c f", p=128, c=NSUB)
    nz_r = nz_d[:, 0].rearrange("(k p) -> k p", p=128)      # (8, 128) contig
    out_r = out_d[:, 0].rearrange("(k p) -> k p", p=128)    # (8, 128) contig

    with tile.TileContext(nc) as tc:
        with (
            tc.tile_pool(name="ypool", bufs=5) as ypool,
            tc.tile_pool(name="xpool", bufs=4) as xpool,
            tc.tile_pool(name="small", bufs=1) as small,
            tc.tile_pool(name="scratch", bufs=2) as scratch,
            tc.tile_pool(name="psum", bufs=1, space="PSUM") as psum,
            tc.tile_pool(name="dram", bufs=1, space="DRAM") as dram,
        ):
            # warm up ncfw + absorb the collective entry rendezvous early,
            # fully overlapped with the streams
            warm = small.tile([1, 8], FP32)
            nc.gpsimd.memset(warm[:], 0.0)
            warm_in = dram.tile([8], FP32)
            warm_out = dram.tile([8], FP32)
            nc.gpsimd.dma_start(warm_in[:], warm[:])
            nc.gpsimd.collective_compute(
                "AllReduce",
                mybir.AluOpType.add,
                replica_groups=[list(range(NCORES))],
                ins=[warm_in.opt()],
                outs=[warm_out.opt()],
            )

            ones128 = small.tile([128, 128], FP32)
            nc.gpsimd.memset(ones128[:], 1.0)

            # ---- phase A: fold each y super-tile as it lands ----
            acc = small.tile([128, FL], FP32)
            for s in range(NST):
                ytile = ypool.tile([128, NSUB, FL], FP32, tag="y")
                nc.sync.dma_start(ytile[:, 0:NSUB // 2, :],
                                  y_r[s, :, 0:NSUB // 2, :])
                nc.scalar.dma_start(ytile[:, NSUB // 2:, :],
                                    y_r[s, :, NSUB // 2:, :])
                # in-place pairwise fold: 8 -> 4 -> 2 subtiles
                nc.vector.tensor_add(ytile[:, 0:4, :], ytile[:, 0:4, :],
                                     ytile[:, 4:8, :])
                nc.vector.tensor_add(ytile[:, 0:2, :], ytile[:, 0:2, :],
                                     ytile[:, 2:4, :])
                if s == 0:
                    nc.vector.tensor_tensor(
                        out=acc[:], in0=ytile[:, 0, :], in1=ytile[:, 1, :],
                        op=mybir.AluOpType.add)
                else:
                    nc.vector.tensor_add(acc[:], acc[:], ytile[:, 0, :])
                    nc.vector.tensor_add(acc[:], acc[:], ytile[:, 1, :])
            # partition-sum + 128-way broadcast in ONE matmul:
            # bc[q, f] = sum_p ones[p, q] * acc[p, f]
            bc = psum.tile([128, FL], FP32, tag="bc")
            nc.tensor.matmul(bc[:], ones128[:], acc[:],
                             start=True, stop=True)

            # ---- phase B: partial dots for ALL rows while x streams ----
            s_part = small.tile([128, NT], FP32)
            for s in range(NST):
                xtile = xpool.tile([128, NSUB, FL], FP32, tag="x")
                nc.sync.dma_start(xtile[:, 0:NSUB // 2, :],
                                  x_r[s, :, 0:NSUB // 2, :])
                nc.scalar.dma_start(xtile[:, NSUB // 2:, :],
                                    x_r[s, :, NSUB // 2:, :])
                for t in range(NSUB):
                    m = s * NSUB + t
                    prod = scratch.tile([128, FL], FP32, tag="sc")
                    nc.vector.scalar_tensor_tensor(
                        out=prod[:],
                        in0=xtile[:, t, :],
                        scalar=1.0,
                        in1=bc[:],
                        op0=mybir.AluOpType.mult,
                        op1=mybir.AluOpType.mult,
                        accum_out=s_part[:, m:m + 1],
                    )

            # ---- transpose s_part (128, 64) -> (64, 128) in 32x32 blocks
            # so the ReduceScatter bounce DMA is contiguous ----
            s_t = small.tile([64, 128], FP32)
            for i in range(4):
                for j in range(2):
                    nc.vector.transpose(
                        s_t[32 * j:32 * (j + 1), 32 * i:32 * (i + 1)],
                        s_part[32 * i:32 * (i + 1), 32 * j:32 * (j + 1)],
                    )

            # ---- ReduceScatter: sum partials, keep our 1024-row shard ----
            cc_in = dram.tile([B], FP32)
            cc_out = dram.tile([BL], FP32)
            nc.gpsimd.dma_start(cc_in[:].rearrange("(m p) -> m p", p=128),
                                s_t[:])
            nc.gpsimd.collective_compute(
                "ReduceScatter",
                mybir.AluOpType.add,
                replica_groups=[list(range(NCORES))],
                ins=[cc_in.opt()],
                outs=[cc_out.opt()],
            )
            s_mine = small.tile([NSUB, 128], FP32)
            nc.gpsimd.dma_start(s_mine[:],
                                cc_out[:].rearrange("(k p) -> k p", p=128))

            # ---- tail: hardswish, + noise, hardtanh (in (8,128) layout) ----
            noise_t = small.tile([NSUB, 128], FP32)
            nc.gpsimd.dma_start(noise_t[:], nz_r)

            t_ = small.tile([NSUB, 128], FP32)
            nc.vector.tensor_scalar(
                out=t_[:], in0=s_mine[:], scalar1=3.0, scalar2=0.0,
                op0=mybir.AluOpType.add, op1=mybir.AluOpType.max,
            )
            nc.vector.tensor_scalar(
                out=t_[:], in0=t_[:], scalar1=6.0, scalar2=1.0 / 6.0,
                op0=mybir.AluOpType.min, op1=mybir.AluOpType.mult,
            )
            r = small.tile([NSUB, 128], FP32)
            nc.vector.tensor_tensor(
                out=r[:], in0=s_mine[:], in1=t_[:], op=mybir.AluOpType.mult,
            )
            nc.vector.tensor_tensor(
                out=r[:], in0=r[:], in1=noise_t[:], op=mybir.AluOpType.add,
            )
            nc.vector.tensor_scalar(
                out=r[:], in0=r[:], scalar1=-0.5, scalar2=0.5,
                op0=mybir.AluOpType.max, op1=mybir.AluOpType.min,
            )
            nc.gpsimd.dma_start(out_r, r[:])

    nc.compile()
    return nc


def _get_nc():
    if "nc" not in _CACHE:
        _CACHE["nc"] = _build()
    return _CACHE["nc"]


# device row (s p c) -> global row 128*(8s+c)+p, so that s_part column
# m = 8s+c, partition p lands on global row 128m+p (what the RS expects)
def _permute_rows(a: np.ndarray) -> np.ndarray:
    # a: (8192, cols); view as (s, c, p, cols), want (s, p, c, cols)
    return np.ascontiguousarray(
        a.reshape(NST, NSUB, 128, a.shape[1]).transpose(0, 2, 1, 3)
        .reshape(B, a.shape[1])
    )


def kernel(x: np.ndarray, y: np.ndarray, noise: np.ndarray, **_run_kwargs) -> np.ndarray:
    x = np.ascontiguousarray(x, dtype=np.float32)
    y = np.ascontiguousarray(y, dtype=np.float32)
    noise = np.ascontiguousarray(noise, dtype=np.float32)

    nc = _get_nc()
    xp = _permute_rows(x)
    in_maps = [
        {
            "x": np.ascontiguousarray(xp[:, i * FL:(i + 1) * FL]),
            "y": np.ascontiguousarray(y[:, i * FL:(i + 1) * FL]),
            "noise": noise[i * BL:(i + 1) * BL],
        }
        for i in range(NCORES)
    ]
    res = run_bass_kernel_spmd(nc, in_maps, list(range(NCORES)), **_run_kwargs)
    out = np.concatenate([res.results[i]["out"] for i in range(NCORES)], axis=0)
    if _run_kwargs:
        _CACHE["last_results"] = res
    return out


# revision 14
# speedup vs baseline: 1.3586x; 1.3586x over previous
"""Trainium2 Bass kernel for nn_Model_1580547969651.

Math (from the reference):
    s    = x @ sum(y, axis=0)          # (B,) row-sums of x @ y^T
    h    = hardswish(s)                # s * clip(s+3, 0, 6) / 6
    out  = clip(h + noise, -0.5, 0.5)  # (B, 1)

Strategy: COLUMN-shard x and y across the 8 cores (512 features each).
Each core's column-sum of its y shard is locally complete, so there is
no mid-kernel collective. y streams in (s p c)-packed so every DMA
descriptor covers a contiguous 16KB per partition; the VectorEngine
folds each 2MB super-tile into a (128, 512) accumulator as it lands.
One ones(128,128) matmul then does the partition-sum AND the 128-way
broadcast in one shot. Phase B computes partial dots s_i = x[:, F_i] @
ysum_i for ALL 8192 rows with fused scalar_tensor_tensor ops while x
streams (host pre-permutes x rows so this layout still produces
batch-ordered partials). The partials are transposed on the
VectorEngine (32x32 blocks) so the ReduceScatter bounce DMA is
contiguous; the 32KB->4KB ReduceScatter hands each core its 1024-row
output shard, and the elementwise tail runs in a DMA-friendly (8,128)
layout. A dummy 32B AllReduce issued up front absorbs ncfw wake-up /
rendezvous costs while the streams run.
"""

import numpy as np

from concourse import bass, bacc, mybir, tile
from concourse.bass_utils import run_bass_kernel_spmd

B = 8192
F = 4096
NCORES = 8
FL = F // NCORES        # 512 features per core
BL = B // NCORES        # 1024 output rows per core
NST = 8                 # y/x super-tiles (128 part x 8 subtiles x 512)
NSUB = 8                # subtiles per super-tile
NT = NST * NSUB         # 64 (128-row) tiles covering all 8192 rows
FP32 = mybir.dt.float32

_CACHE: dict = {}


def _build():
    nc = bacc.Bacc(
        "TRN2",
        target_bir_lowering=False,
        debug=False,
        num_devices=NCORES,
    )

    x_d = nc.dram_tensor("x", [B, FL], FP32, kind="ExternalInput")
    y_d = nc.dram_tensor("y", [B, FL], FP32, kind="ExternalInput")
    nz_d = nc.dram_tensor("noise", [BL, 1], FP32, kind="ExternalInput")
    out_d = nc.dram_tensor("out", [BL, 1], FP32, kind="ExternalOutput")

    # (s p c) packing: partition p's slice of super-tile s is 8 consecutive
    # DRAM rows = one contiguous 16KB chunk per descriptor.
    y_r = y_d[:, :].rearrange("(s p c) f -> s p c f", p=128, c=NSUB)
    x_r = x_d[:, :].rearrange("(s p c) f -> s p c f", p=128, c=NSUB)
    nz_r = nz_d[:, 0].rearrange("(k p) -> k p", p=128)      # (8, 128) contig
    out_r = out_d[:, 0].rearrange("(k p) -> k p", p=128)    # (8, 128) contig

    with tile.TileContext(nc) as tc:
        with (
            tc.tile_pool(name="ypool", bufs=5) as ypool,
            tc.tile_pool(name="xpool", bufs=4) as xpool,
            tc.tile_pool(name="small", bufs=1) as small,
            tc.tile_pool(name="scratch", bufs=2) as scratch,
            tc.tile_pool(name="psum", bufs=1, space="PSUM") as psum,
            tc.tile_pool(name="dram", bufs=1, space="DRAM") as dram,
        ):
            ones128 = small.tile([128, 128], FP32)
            nc.gpsimd.memset(ones128[:], 1.0)

            # noise is only needed at the very end; load it now so the
            # gpsimd queue isn't fetching it behind the ReduceScatter
            noise_t = small.tile([NSUB, 128], FP32)
            nc.gpsimd.dma_start(noise_t[:], nz_r)

            # ---- phase A: fold each y super-tile as it lands ----
            acc = small.tile([128, FL], FP32)
            for s in range(NST):
                ytile = ypool.tile([128, NSUB, FL], FP32, tag="y")
                nc.sync.dma_start(ytile[:, 0:NSUB // 2, :],
                                  y_r[s, :, 0:NSUB // 2, :])
                nc.scalar.dma_start(ytile[:, NSUB // 2:, :],
                                    y_r[s, :, NSUB // 2:, :])
                # in-place pairwise fold: 8 -> 4 -> 2 subtiles
                nc.vector.tensor_add(ytile[:, 0:4, :], ytile[:, 0:4, :],
                                     ytile[:, 4:8, :])
                nc.vector.tensor_add(ytile[:, 0:2, :], ytile[:, 0:2, :],
                                     ytile[:, 2:4, :])
                if s == 0:
                    nc.vector.tensor_tensor(
                        out=acc[:], in0=ytile[:, 0, :], in1=ytile[:, 1, :],
                        op=mybir.AluOpType.add)
                else:
                    nc.vector.tensor_add(acc[:], acc[:], ytile[:, 0, :])
                    nc.vector.tensor_add(acc[:], acc[:], ytile[:, 1, :])
            # partition-sum + 128-way broadcast in ONE matmul:
            # bc_ps[q, f] = sum_p ones[p, q] * acc[p, f]
            bc_ps = psum.tile([128, FL], FP32, tag="bc")
            nc.tensor.matmul(bc_ps[:], ones128[:], acc[:],
                             start=True, stop=True)
            # GpSimd can't read PSUM; give both engines an SBUF copy
            bc = small.tile([128, FL], FP32)
            nc.vector.tensor_copy(bc[:], bc_ps[:])

            # ---- phase B: partial dots for ALL rows while x streams ----
            s_part = small.tile([128, NT], FP32)
            for s in range(NST):
                xtile = xpool.tile([128, NSUB, FL], FP32, tag="x")
                nc.sync.dma_start(xtile[:, 0:NSUB // 2, :],
                                  x_r[s, :, 0:NSUB // 2, :])
                nc.scalar.dma_start(xtile[:, NSUB // 2:, :],
                                    x_r[s, :, NSUB // 2:, :])
                for t in range(NSUB):
                    m = s * NSUB + t
                    prod = scratch.tile([128, FL], FP32, tag="sc")
                    nc.vector.scalar_tensor_tensor(
                        out=prod[:],
                        in0=xtile[:, t, :],
                        scalar=1.0,
                        in1=bc[:],
                        op0=mybir.AluOpType.mult,
                        op1=mybir.AluOpType.mult,
                        accum_out=s_part[:, m:m + 1],
                    )

            # ---- transpose s_part (128, 64) -> (64, 128) in 32x32 blocks
            # so the ReduceScatter bounce DMA is contiguous ----
            s_t = small.tile([64, 128], FP32)
            for i in range(4):
                for j in range(2):
                    nc.vector.transpose(
                        s_t[32 * j:32 * (j + 1), 32 * i:32 * (i + 1)],
                        s_part[32 * i:32 * (i + 1), 32 * j:32 * (j + 1)],
                    )

            # ---- ReduceScatter: sum partials, keep our 1024-row shard ----
            cc_in = dram.tile([B], FP32)
            cc_out = dram.tile([BL], FP32)
            nc.gpsimd.dma_start(cc_in[:].rearrange("(m p) -> m p", p=128),
                                s_t[:])
            nc.gpsimd.collective_compute(
                "ReduceScatter",
                mybir.AluOpType.add,
                replica_groups=[list(range(NCORES))],
                ins=[cc_in.opt()],
                outs=[cc_out.opt()],
            )
            s_mine = small.tile([NSUB, 128], FP32)
            nc.gpsimd.dma_start(s_mine[:],
                                cc_out[:].rearrange("(k p) -> k p", p=128))

            # ---- tail: hardswish, + noise, hardtanh (in (8,128) layout) ----
            t_ = small.tile([NSUB, 128], FP32)
            nc.vector.tensor_scalar(
                out=t_[:], in0=s_mine[:], scalar1=3.0, scalar2=0.0,
                op0=mybir.AluOpType.add, op1=mybir.AluOpType.max,
            )
            nc.vector.tensor_scalar(
                out=t_[:], in0=t_[:], scalar1=6.0, scalar2=1.0 / 6.0,
                op0=mybir.AluOpType.min, op1=mybir.AluOpType.mult,
            )
            r = small.tile([NSUB, 128], FP32)
            nc.vector.tensor_tensor(
                out=r[:], in0=s_mine[:], in1=t_[:], op=mybir.AluOpType.mult,
            )
            nc.vector.tensor_tensor(
                out=r[:], in0=r[:], in1=noise_t[:], op=mybir.AluOpType.add,
            )
            nc.vector.tensor_scalar(
                out=r[:], in0=r[:], scalar1=-0.5, scalar2=0.5,
                op0=mybir.AluOpType.max, op1=mybir.AluOpType.min,
            )
            nc.gpsimd.dma_start(out_r, r[:])

    nc.compile()
    return nc


def _get_nc():
    if "nc" not in _CACHE:
        _CACHE["nc"] = _build()
    return _CACHE["nc"]


# device row (s p c) -> global row 128*(8s+c)+p, so that s_part column
# m = 8s+c, partition p lands on global row 128m+p (what the RS expects)
def _permute_rows(a: np.ndarray) -> np.ndarray:
    # a: (8192, cols); view as (s, c, p, cols), want (s, p, c, cols)
    return np.ascontiguousarray(
        a.reshape(NST, NSUB, 128, a.shape[1]).transpose(0, 2, 1, 3)
        .reshape(B, a.shape[1])
    )


def kernel(x: np.ndarray, y: np.ndarray, noise: np.ndarray, **_run_kwargs) -> np.ndarray:
    x = np.ascontiguousarray(x, dtype=np.float32)
    y = np.ascontiguousarray(y, dtype=np.float32)
    noise = np.ascontiguousarray(noise, dtype=np.float32)

    nc = _get_nc()
    xp = _permute_rows(x)
    in_maps = [
        {
            "x": np.ascontiguousarray(xp[:, i * FL:(i + 1) * FL]),
            "y": np.ascontiguousarray(y[:, i * FL:(i + 1) * FL]),
            "noise": noise[i * BL:(i + 1) * BL],
        }
        for i in range(NCORES)
    ]
    res = run_bass_kernel_spmd(nc, in_maps, list(range(NCORES)), **_run_kwargs)
    out = np.concatenate([res.results[i]["out"] for i in range(NCORES)], axis=0)
    if _run_kwargs:
        _CACHE["last_results"] = res
    return out
